# revision 2
# baseline (speedup 1.0000x reference)
"""Trainium2 Bass kernel for 16-head MHA (B=4, S=2048, D=1024, H=16) — v2.

Sharding (8 NeuronCores, SPMD, no collectives):
  - DP=4 over batch: core c handles batch c//2
  - TP=2 over heads: t = c%2 handles heads [8t..8t+8) == QKV out dims
    [512t..512t+512)  (column-parallel QKV, row-parallel O)
  - host: slices inputs, pre-transposes + casts to bf16, sums the 2
    O-projection partials per batch, adds bo.

Per-core kernel (bf16 matmuls, fp32 PSUM):
  - Projections -> QT / KT2 (dk-major, head-pair packed: even head in
    partitions 0:64, odd head in 64:128 — no zero padding) and V1
    (token-major, 65 cols per head incl. ones column for the softmax
    denominator).
  - Scores ROW-TILED: per k-tile two concurrent contract-64 matmuls at
    tile_position (0,0)/(64,0) fill one [128,1024] PSUM group (head A
    cols 0:512, head B 512:1024).  Full-array activity keeps HAM at
    2.4 GHz without v1's zero-padding.
  - ONE exp per k-tile over the [128,1024] group (amortizes the ~293ns
    ACT instruction overhead across both heads).
  - attn@V: stationary exp tile, rhs V1[128,65]; DVE recip+scale; PE
    transpose per q-tile pair into dk-major xattT; O-projection.
  - 16 units (qb, hp) x 16 k-tiles; filler chains (projections, attn@V
    of the previous unit, O-proj) are popped between k-tiles so PE and
    ScalarE (exp, the steady-state bottleneck) both stay fed.
  - xk/xv are DMA'd twice (pass 1 for head-pair chunks t0/t1 resp. V
    halves 0, pass 2 for t2/t3 / half 1) so chunk-slot rings of 2 fit
    SBUF without cross-unit slot deadlocks; QT and xattT are 2-deep
    qb-windowed rings.
"""

import numpy as np

P = 128
B, S, D, H = 4, 2048, 1024, 16
DK = 64
H_SH = 8                    # heads per core (TP=2)
HP = H_SH // 2              # 4 head-pairs per core
DSH = H_SH * DK             # 512 qkv out dims per core
TOK = S                     # 2048 tokens per core (1 batch)
DC = D // P                 # 8 contraction chunks
TB = 512                    # token block for projections
NTB = TOK // TB             # 4
KT = S // P                 # 16 key tiles
QB = 512                    # q block per unit
NQB = S // QB               # 4
VA = H_SH * (DK + 1)        # 520 = V width incl. ones columns
VH = VA // 2                # 260

_CACHE = {}


def _build_nc(bias_v=False):
    import concourse.tile as tile
    from concourse import bacc, mybir
    from concourse.masks import make_identity

    bf16 = mybir.dt.bfloat16
    fp32 = mybir.dt.float32

    nc = bacc.Bacc("TRN2", target_bir_lowering=False, debug=False)

    xqT = nc.dram_tensor("xqT", [D, TOK], bf16, kind="ExternalInput").ap()
    xkT = nc.dram_tensor("xkT", [D, TOK], bf16, kind="ExternalInput").ap()
    xvT = nc.dram_tensor("xvT", [D, TOK], bf16, kind="ExternalInput").ap()
    wqT = nc.dram_tensor("wqT", [D, DSH], bf16, kind="ExternalInput").ap()
    wkT = nc.dram_tensor("wkT", [D, DSH], bf16, kind="ExternalInput").ap()
    wvT = nc.dram_tensor("wvT", [D, VA], bf16, kind="ExternalInput").ap()
    woT = nc.dram_tensor("woT", [DSH, D], bf16, kind="ExternalInput").ap()
    bq_d = nc.dram_tensor("bq_s", [DSH], fp32, kind="ExternalInput").ap()
    bk_d = nc.dram_tensor("bk_s", [DSH], fp32, kind="ExternalInput").ap()
    bv_d = nc.dram_tensor("bv_a", [VA], bf16, kind="ExternalInput").ap()
    y = nc.dram_tensor("y", [TOK, D], fp32, kind="ExternalOutput").ap()

    xkT_r = xkT.rearrange("(c p) t -> p c t", p=P)
    xqT_r = xqT.rearrange("(c p) t -> p c t", p=P)
    xvT_r = xvT.rearrange("(c p) t -> p c t", p=P)

    with tile.TileContext(nc) as tc:
        from contextlib import ExitStack

        with ExitStack() as ctx:
            singles = ctx.enter_context(tc.tile_pool(name="singles", bufs=1))

            wk_sb = singles.tile([P, DC, DSH], bf16)
            wq_sb = singles.tile([P, DC, DSH], bf16)
            wkT_r = wkT.rearrange("(c p) e -> p c e", p=P)
            wqT_r2 = wqT.rearrange("(c p) e -> p c e", p=P)
            bq_sb = singles.tile([P, DSH // P], fp32)
            nc.scalar.dma_start(out=bq_sb, in_=bq_d.rearrange("(t p) -> p t", p=P))
            bk_sb = singles.tile([P, DSH // P], fp32)
            nc.scalar.dma_start(out=bk_sb, in_=bk_d.rearrange("(t p) -> p t", p=P))
            bv_sb = singles.tile([1, VA], bf16)
            nc.scalar.dma_start(out=bv_sb, in_=bv_d.rearrange("(a e) -> a e", a=1))
            wv_sb = singles.tile([P, DC, VA], bf16)
            wo_sb = singles.tile([P, DSH // P, D], bf16)
            ones_sb = singles.tile([1, P], bf16)
            nc.vector.memset(ones_sb, 1.0)
            ident = singles.tile([P, P], bf16)
            make_identity(nc, ident[:])

            # qb-windowed rings (2 deep over qb)
            QT_sb = singles.tile([P, HP, 2, QB], bf16)
            xattT = singles.tile([P, HP, 2, QB], bf16)
            KT2 = singles.tile([P, HP, TOK], bf16)
            V1_sb = singles.tile([P, KT, VA], bf16)

            import concourse.mybir as mybir2

            with tc.tile_pool(name="xk_p", bufs=2) as xk_pool, \
                 tc.tile_pool(name="xq_p", bufs=2) as xq_pool, \
                 tc.tile_pool(name="xv_p", bufs=2) as xv_pool, \
                 tc.tile_pool(name="exps", bufs=2) as exps_pool, \
                 tc.tile_pool(name="small", bufs=4) as small_pool, \
                 tc.tile_pool(name="ysb", bufs=2) as y_pool, \
                 tc.tile_pool(name="st_p", bufs=2, space="PSUM") as st_pool, \
                 tc.tile_pool(name="av_p", bufs=2, space="PSUM") as av_pool, \
                 tc.tile_pool(name="fl_p", bufs=2, space="PSUM") as fl_pool:

                if not bias_v:
                    nc.vector.memset(
                        V1_sb.rearrange("p k (h w) -> p k h w",
                                        w=DK + 1)[:, :, :, DK], 1.0)

                # ---- DMA groups (emission order == queue order) ----------
                # Per-chunk DMAs (chunks parallelize across DMA engines;
                # whole-group strided DMAs measured ~2x slower).  Block-0
                # wk/xk chunks interleave as (wk-c, xk-c) pairs on sync and
                # wq/xq0 pairs on scalar, so the first K/Q chains unblock
                # chunk-by-chunk.  kg1[1] rides the otherwise-idle scalar
                # queue (fresh ring slot -> no WAR can stall ScalarE).
                def k_group(tb, eng):
                    sl = xk_pool.tile([P, DC, TB], bf16, tag="k", name="kgt")
                    for c in range(DC):
                        eng.dma_start(out=sl[:, c],
                                      in_=xkT_r[:, c, tb * TB:(tb + 1) * TB])
                    return sl

                def q_group(tb, eng):
                    sl = xq_pool.tile([P, DC, TB], bf16, tag="q", name="qgt")
                    for c in range(DC):
                        eng.dma_start(out=sl[:, c],
                                      in_=xqT_r[:, c, tb * TB:(tb + 1) * TB])
                    return sl

                def v_group(tb, eng):
                    sl = xv_pool.tile([P, DC, TB], bf16, tag="v", name="vgt")
                    for c in range(DC):
                        eng.dma_start(out=sl[:, c],
                                      in_=xvT_r[:, c, tb * TB:(tb + 1) * TB])
                    return sl

                kg0 = xk_pool.tile([P, DC, TB], bf16, tag="k", name="kg0")
                qg0 = xq_pool.tile([P, DC, TB], bf16, tag="q", name="qg0")
                for c in range(DC):
                    nc.sync.dma_start(out=wk_sb[:, c], in_=wkT_r[:, c])
                    nc.sync.dma_start(out=kg0[:, c],
                                      in_=xkT_r[:, c, 0:TB])
                    nc.scalar.dma_start(out=wq_sb[:, c], in_=wqT_r2[:, c])
                    nc.scalar.dma_start(out=qg0[:, c],
                                        in_=xqT_r[:, c, 0:TB])
                kg1 = [kg0, k_group(1, nc.scalar)]
                kg1.append(k_group(2, nc.sync))
                kg1.append(k_group(3, nc.sync))
                kg2 = [k_group(0, nc.sync), k_group(1, nc.sync)]
                for c in range(DC):
                    nc.gpsimd.dma_start(
                        out=wv_sb[:, c],
                        in_=wvT.rearrange("(c p) e -> p c e", p=P)[:, c])
                vg1 = [v_group(0, nc.gpsimd), v_group(1, nc.gpsimd),
                       v_group(2, nc.gpsimd), v_group(3, nc.gpsimd)]
                kg2.append(k_group(2, nc.sync))
                kg2.append(k_group(3, nc.sync))
                vg2 = [v_group(0, nc.gpsimd), v_group(1, nc.gpsimd),
                       v_group(2, nc.gpsimd), v_group(3, nc.gpsimd)]
                qg = [qg0, q_group(1, nc.sync)]
                nc.sync.dma_start(
                    out=wo_sb, in_=woT.rearrange("(t p) e -> p t e", p=P))
                qg.append(q_group(2, nc.gpsimd))
                qg.append(q_group(3, nc.gpsimd))

                # ---- chains ----------------------------------------------
                def k_chain(tb, t, sl):
                    def f():
                        ps = fl_pool.tile([P, TB], fp32, tag="f")
                        for c in range(DC):
                            nc.tensor.matmul(
                                ps, lhsT=wk_sb[:, c, t * P:(t + 1) * P],
                                rhs=sl[:, c], start=(c == 0),
                                stop=(c == DC - 1))
                        tl = tb * TB
                        nc.vector.tensor_scalar_add(
                            KT2[0:DK, t, tl:tl + TB], ps[0:DK],
                            bk_sb[0:DK, t:t + 1])
                        nc.vector.tensor_scalar_add(
                            KT2[DK:P, t, tl:tl + TB], ps[DK:P],
                            bk_sb[DK:P, t:t + 1])
                    return f

                def q_chain(qb, t):
                    def f():
                        ps = fl_pool.tile([P, TB], fp32, tag="f")
                        for c in range(DC):
                            nc.tensor.matmul(
                                ps, lhsT=wq_sb[:, c, t * P:(t + 1) * P],
                                rhs=qg[qb][:, c], start=(c == 0),
                                stop=(c == DC - 1))
                        nc.vector.tensor_scalar_add(
                            QT_sb[:, t, qb % 2, :], ps, bq_sb[:, t:t + 1])
                    return f

                def v_chain(tb, i, half, sl):
                    def f():
                        ps = fl_pool.tile([P, VH], fp32, tag="f")
                        c0 = half * VH
                        for c in range(DC):
                            nc.tensor.matmul(
                                ps, lhsT=sl[:, c, i * P:(i + 1) * P],
                                rhs=wv_sb[:, c, c0:c0 + VH],
                                start=(c == 0),
                                stop=(not bias_v and c == DC - 1))
                        if bias_v:
                            nc.tensor.matmul(
                                ps, lhsT=ones_sb, rhs=bv_sb[:, c0:c0 + VH],
                                start=False, stop=True)
                            nc.vector.tensor_copy(
                                out=V1_sb[:, tb * 4 + i, c0:c0 + VH], in_=ps)
                        else:
                            nc.vector.tensor_copy(
                                out=V1_sb.rearrange(
                                    "p k (h w) -> p k h w",
                                    w=DK + 1)[:, tb * 4 + i,
                                              half * 4:(half + 1) * 4, 0:DK],
                                in_=ps.rearrange(
                                    "p (h w) -> p h w", w=DK + 1)[:, :, 0:DK])
                    return f

                def oproj_chain(qb, ot, nck):
                    tok0 = qb * QB + ot * P

                    def f():
                        y_ps = fl_pool.tile([P, 512], fp32, tag="f")
                        for t2 in range(DSH // P):
                            nc.tensor.matmul(
                                y_ps,
                                lhsT=xattT[:, t2, qb % 2, ot * P:(ot + 1) * P],
                                rhs=wo_sb[:, t2, nck * 512:(nck + 1) * 512],
                                start=(t2 == 0), stop=(t2 == DSH // P - 1))
                        y_sb = y_pool.tile([P, 512], fp32, tag="y")
                        nc.vector.tensor_copy(out=y_sb, in_=y_ps)
                        nc.sync.dma_start(
                            out=y[tok0:tok0 + P, nck * 512:(nck + 1) * 512],
                            in_=y_sb)
                    return f

                def attn_av_chain(hp, head, qb, exp_t, qt, pair):
                    h = 2 * hp + head
                    att_ps = av_pool.tile([P, DK + 1], fp32, tag="a")
                    for kt in range(KT):
                        nc.tensor.matmul(
                            att_ps,
                            lhsT=exp_t[:, kt,
                                       head * QB + qt * P:
                                       head * QB + (qt + 1) * P],
                            rhs=V1_sb[:, kt, h * (DK + 1):(h + 1) * (DK + 1)],
                            start=(kt == 0), stop=(kt == KT - 1))
                    recip = small_pool.tile([P, 1], fp32, tag="recip")
                    nc.vector.reciprocal(recip, att_ps[:, DK:DK + 1])
                    half = (qt % 2) * DK
                    nc.vector.tensor_scalar_mul(
                        pair[:, half:half + DK], att_ps[:, 0:DK], recip)
                    if qt % 2 == 1:
                        tp = fl_pool.tile([P, P], bf16, tag="f")
                        nc.tensor.transpose(tp, pair, ident)
                        dko = head * DK
                        tok0 = (qt - 1) * P
                        nc.vector.tensor_copy(
                            out=xattT[dko:dko + DK, hp, qb % 2,
                                      tok0:tok0 + P],
                            in_=tp[0:DK])
                        nc.vector.tensor_copy(
                            out=xattT[dko:dko + DK, hp, qb % 2,
                                      tok0 + P:tok0 + 2 * P],
                            in_=tp[DK:P])

                def av_chains(hp, qb, exp_t):
                    st = {}
                    chains = []
                    for head in range(2):
                        for qt in range(QB // P):
                            def f(head=head, qt=qt):
                                if qt % 2 == 0:
                                    st['pair'] = small_pool.tile(
                                        [P, P], bf16, tag="xatt",
                                        name="pair")
                                attn_av_chain(hp, head, qb, exp_t, qt,
                                              st['pair'])
                            chains.append(f)
                    return chains

                def attn_unit(hp, qb, fillers):
                    exp_t = exps_pool.tile([P, KT, 2 * QB], bf16, tag="exps")
                    for kt in range(KT):
                        stq = st_pool.tile([P, 2 * QB], fp32, tag="st")
                        kl = kt * P
                        nc.tensor.matmul(
                            stq[:, 0:QB],
                            lhsT=KT2[0:DK, hp, kl:kl + P],
                            rhs=QT_sb[0:DK, hp, qb % 2, :],
                            start=True, stop=True)
                        nc.tensor.matmul(
                            stq[:, QB:2 * QB],
                            lhsT=KT2[DK:P, hp, kl:kl + P],
                            rhs=QT_sb[DK:P, hp, qb % 2, :],
                            start=True, stop=True)
                        nc.scalar.activation(
                            out=exp_t[:, kt, :], in_=stq,
                            func=mybir2.ActivationFunctionType.Exp,
                            scale=0.125)
                        if fillers:
                            fillers.pop(0)()
                    while fillers:
                        fillers.pop(0)()
                    return av_chains(hp, qb, exp_t)

                # ---- upfront chains --------------------------------------
                k_chain(0, 0, kg1[0])()
                q_chain(0, 0)()

                # ---- filler plan -----------------------------------------
                vch = {(tb, i, h): v_chain(tb, i, h, (vg1 if h == 0 else
                                                      vg2)[tb])
                       for tb in range(NTB) for i in range(4)
                       for h in range(2)}

                def vblock(tb, h):
                    return [vch[(tb, i, h)] for i in range(4)]

                fills = {}
                # V(b0/b1) chains interleave among the K chains so the
                # xv ring slots free earlier (their WAR paced the later
                # xv DMAs and with them units 1-3 in the measured trace)
                fills[0] = ([k_chain(1, 0, kg1[1]), k_chain(0, 1, kg1[0]),
                             vch[(0, 0, 0)],
                             k_chain(2, 0, kg1[2]), vch[(0, 1, 0)],
                             k_chain(1, 1, kg1[1]), vch[(0, 2, 0)],
                             k_chain(3, 0, kg1[3]), vch[(0, 3, 0)],
                             k_chain(2, 1, kg1[2]), vch[(1, 0, 0)],
                             k_chain(3, 1, kg1[3]), vch[(1, 1, 0)],
                             vch[(1, 2, 0)], vch[(1, 3, 0)],
                             q_chain(0, 1)])
                fills[1] = (vblock(2, 0)
                            + [k_chain(0, 2, kg2[0]), k_chain(0, 3, kg2[0])]
                            + vblock(3, 0)
                            + [k_chain(1, 2, kg2[1]), k_chain(1, 3, kg2[1]),
                               q_chain(0, 2)])
                fills[2] = ([k_chain(2, 2, kg2[2]), k_chain(2, 3, kg2[2]),
                             k_chain(3, 2, kg2[3]), k_chain(3, 3, kg2[3])]
                            + vblock(0, 1) + vblock(1, 1)
                            + [q_chain(0, 3)])
                # u2: its own K(t2/t3) chains must precede av(u1) so the
                # kt>=8 score matmuls aren't emitted before their producers
                fills[3] = (vblock(2, 1) + vblock(3, 1) + [q_chain(1, 0)])
                for u in range(4, 15):
                    qb_n, hp_n = divmod(u + 1, 4)
                    fills[u] = [q_chain(qb_n, hp_n)]
                fills[15] = []
                # O-proj(qb) chains ride units 4(qb+1)+1 .. +3
                for qb in range(3):
                    och = [oproj_chain(qb, ot, nck)
                           for ot in range(4) for nck in range(2)]
                    base = 4 * (qb + 1)
                    fills[base + 1] += och[0:3]
                    fills[base + 2] += och[3:6]
                    fills[base + 3] += och[6:8]

                # ---- main loop -------------------------------------------
                tail_av = []
                for u in range(16):
                    qb, hp = divmod(u, 4)
                    if u == 1:
                        # V(b2/b3,h0) must precede av(u0) (V1 RAW)
                        flist = fills[1][:10] + tail_av + fills[1][10:]
                    elif u == 2:
                        # K(t2/t3) chains precede av(u1) (KT2 RAW for kt>=8)
                        flist = fills[2][:4] + tail_av + fills[2][4:]
                    elif u == 3:
                        # V(b2/b3,h1) must precede av(u2)
                        flist = fills[3][:8] + tail_av + fills[3][8:]
                    else:
                        flist = tail_av + fills[u]
                    tail_av = attn_unit(hp, qb, flist)

                # tail: av(u15) + O-proj(qb3).  O(qb3, ot) needs BOTH heads'
                # transposes covering its token range, so run av chains
                # grouped by q-tile pair: (h0 q0,q1, h1 q0,q1) -> O(ot 0,1),
                # then (h0 q2,q3, h1 q2,q3) -> O(ot 2,3).
                och = [oproj_chain(3, ot, nck)
                       for ot in range(4) for nck in range(2)]
                for idx in (0, 1, 4, 5):
                    tail_av[idx]()
                for oc in och[0:4]:
                    oc()
                for idx in (2, 3, 6, 7):
                    tail_av[idx]()
                for oc in och[4:8]:
                    oc()

    nc.compile()
    return nc


def _get_nc(bias_v=False):
    key = ("nc", bias_v)
    if key not in _CACHE:
        _CACHE[key] = _build_nc(bias_v)
    return _CACHE[key]


def _prep_inputs(q, k, v, wq, bq, wk, bk, wv, bv, wo):
    import ml_dtypes

    bf16 = ml_dtypes.bfloat16
    in_maps = []
    acts = []
    for b in range(B):
        acts.append(tuple(
            np.ascontiguousarray(np.asarray(x[b]).T).astype(bf16)
            for x in (q, k, v)))
    wslices = []
    for t in range(2):
        sl = slice(t * DSH, (t + 1) * DSH)
        wq_s = np.ascontiguousarray(wq[sl, :].T).astype(bf16)       # (D, DSH)
        wk_s = np.ascontiguousarray(wk[sl, :].T).astype(bf16)
        wv_s = wv[sl, :]                                            # (DSH, D)
        wv_aug = np.zeros((D, VA), np.float32)
        bv_aug = np.zeros(VA, np.float32)
        for hh in range(H_SH):
            wv_aug[:, hh * (DK + 1):hh * (DK + 1) + DK] = \
                wv_s[hh * DK:(hh + 1) * DK, :].T
            bv_aug[hh * (DK + 1):hh * (DK + 1) + DK] = \
                bv[sl][hh * DK:(hh + 1) * DK]
            bv_aug[hh * (DK + 1) + DK] = 1.0
        wo_s = np.ascontiguousarray(wo[:, sl].T).astype(bf16)       # (DSH, D)
        wslices.append((
            wq_s, wk_s, wv_aug.astype(bf16), wo_s,
            np.ascontiguousarray(bq[sl]).astype(np.float32),
            np.ascontiguousarray(bk[sl]).astype(np.float32),
            bv_aug.astype(bf16)))
    for c in range(8):
        b, t = c // 2, c % 2
        xq_s, xk_s, xv_s = acts[b]
        wq_s, wk_s, wv_a, wo_s, bq_s, bk_s, bv_a = wslices[t]
        in_maps.append({
            "xqT": xq_s, "xkT": xk_s, "xvT": xv_s,
            "wqT": wq_s, "wkT": wk_s, "wvT": wv_a, "woT": wo_s,
            "bq_s": bq_s, "bk_s": bk_s, "bv_a": bv_a,
        })
    return in_maps


def _combine(results, bo):
    out = np.zeros((B, S, D), np.float32)
    for b in range(B):
        out[b] = results[2 * b]["y"].astype(np.float32) + \
            results[2 * b + 1]["y"]
    out += np.asarray(bo, np.float32)[None, None, :]
    return out


def kernel_with_results(q, k, v, mask, wq, bq, wk, bk, wv, bv, wo, bo,
                        trace=False):
    from concourse.bass_utils import run_bass_kernel_spmd

    q, k, v = np.asarray(q), np.asarray(k), np.asarray(v)
    wq, bq = np.asarray(wq), np.asarray(bq)
    wk, bk = np.asarray(wk), np.asarray(bk)
    wv, bv = np.asarray(wv), np.asarray(bv)
    wo, bo = np.asarray(wo), np.asarray(bo)
    mask = np.asarray(mask)
    if not mask.all():
        return _host_reference(q, k, v, mask, wq, bq, wk, bk, wv, bv,
                               wo, bo), None

    nc = _get_nc(bias_v=bool(np.any(bv)))
    in_maps = _prep_inputs(q, k, v, wq, bq, wk, bk, wv, bv, wo)
    res = run_bass_kernel_spmd(nc, in_maps, core_ids=list(range(8)),
                               trace=trace)
    return _combine(res.results, bo), res


def kernel(**inputs):
    out, _ = kernel_with_results(**inputs)
    return out


def _host_reference(q, k, v, mask, wq, bq, wk, bk, wv, bv, wo, bo):
    def proj(x, w, b):
        return np.einsum("bsd,ed->bse", x, w) + b

    def split_heads(x):
        return x.reshape(B, S, H, DK).transpose(0, 2, 1, 3)

    qh = split_heads(proj(q, wq, bq))
    kh = split_heads(proj(k, wk, bk))
    vh = split_heads(proj(v, wv, bv))
    scores = np.einsum("bhqd,bhkd->bhqk", qh, kh) / np.sqrt(np.float32(DK))
    scores = np.where(mask == 0, np.float32(-1e9), scores)
    scores -= scores.max(-1, keepdims=True)
    e = np.exp(scores)
    attn = e / e.sum(-1, keepdims=True)
    x = np.einsum("bhqk,bhkd->bhqd", attn, vh)
    x = x.transpose(0, 2, 1, 3).reshape(B, S, D)
    return np.einsum("bsd,ed->bse", x, wo) + bo
